# revision 11
# baseline (speedup 1.0000x reference)
"""Trainium2 Bass kernel for a dense transformer block (B=4, T=2048, C=1024, H=16, FF=4096).

Contract: kernel(**inputs) takes FULL fp32 numpy inputs (as produced by the
reference setup_inputs) and returns the FULL [4, 2048, 1024] fp32 output.

Sharding: 8 cores = (batch b in 0..3) x (T-half j in 0..1). Uniform SPMD
program; causal masking is handled by per-core bias data (context tiles) plus
static triangular masks (diagonal tiles), with the per-core KV token order
permuted on the host so every core sees the same static structure.

All on-chip activations are stored transposed ([feature, token]) so every
matmul contracts along SBUF partitions with no transposes anywhere. The
softmax denominator is computed for free as a 65th "ones" column of V'.
"""

import functools
import os

import numpy as np
import ml_dtypes

os.environ.setdefault("JAX_COMPILATION_CACHE_DIR", "/tmp/jax_kernel_cache")

# ---- problem constants (hardcoded per the harness contract) ----
B, T, C = 4, 2048, 1024
H, HS, FF = 16, 64, 4096
P = 128                    # SBUF partitions
TQ = T // 2                # queries per core (1024)
NTQ = TQ // 512            # tq 512-tiles (2)
NS = T // P                # s 128-tiles (16)
NPAIR = H // 2             # head pairs (8)
KC = C // P                # c 128-tiles (8)
NF = FF // P               # f 128-tiles (32)
NCO = C // P               # output-channel 128-tiles (8)
NCORE = 8
SCALE = C ** -0.5          # softmax scale (1/32)
NEG = -1e9

BF16 = ml_dtypes.bfloat16


def _tri_masks_np():
    """tri[s, m, q] = 1.0 if q >= 128*m + s else 0, shaped [128, 4, 512], bf16."""
    s = np.arange(P)[:, None, None]
    m = np.arange(4)[None, :, None]
    q = np.arange(512)[None, None, :]
    return (q >= P * m + s).astype(BF16)


def build_body(nc, xkv, xres, kbias, wq, wk, wv, wproj, w1, w2, bproj, b1, b2, out):
    """Emit the per-core kernel body. All args are DRAM tensor handles/APs.

    xkv:   [C, T]      bf16  x_b^T, KV-token order (own half last)
    xres:  [P, KC, TQ] f32   own-half x^T (partition-tiled) for the residual
    kbias: [P, NTQ, NS] f32  per (tq-tile, s-tile) additive mask bias
    wq/wk/wv: [C, C]   bf16  head-major columns (h*64+d)
    wproj: [C, C] bf16; w1: [C, FF] bf16; w2: [FF, C] bf16
    bproj/b2: [P, NCO] f32; b1: [P, NF] f32   (col k = bias[k*128:(k+1)*128])
    out:   [C, TQ]     f32   transposed output for this core's token chunk
    """
    import concourse.tile as tile
    import concourse.mybir as mybir
    from contextlib import ExitStack

    f32 = mybir.dt.float32
    bf16 = mybir.dt.bfloat16
    Exp = mybir.ActivationFunctionType.Exp
    Relu = mybir.ActivationFunctionType.Relu
    Copy = mybir.ActivationFunctionType.Copy

    tri_dram = nc.inline_tensor(_tri_masks_np(), name="tri_masks")

    with tile.TileContext(nc) as tc, ExitStack() as ctx:
        # ---------------- persistent pools ----------------
        persist = ctx.enter_context(tc.tile_pool(name="persist", bufs=1))

        kb_sb = persist.tile([P, NTQ, NS], f32, tag="kbias")
        nc.sync.dma_start(out=kb_sb, in_=kbias)

        tri_sb = persist.tile([P, 4, 512], bf16, tag="tri")
        nc.sync.dma_start(out=tri_sb, in_=tri_dram[:])

        # pools scoped to free SBUF between phases (closed in LIFO order)
        attn_ctx = ExitStack()
        attn_pool = attn_ctx.enter_context(tc.tile_pool(name="attn_pool", bufs=1))
        yt_ctx = ExitStack()
        yt_pool = yt_ctx.enter_context(tc.tile_pool(name="yt_pool", bufs=1))
        xkv_ctx = ExitStack()
        xkv_pool = xkv_ctx.enter_context(tc.tile_pool(name="xkv_pool", bufs=1))

        xkv_sb = xkv_pool.tile([P, KC, T], bf16, tag="xkv")
        nc.sync.dma_start(out=xkv_sb, in_=xkv.rearrange("(kc p) s -> p kc s", p=P))

        qt_sb = [attn_pool.tile([P, TQ], bf16, tag=f"qt{p}", name=f"qt{p}") for p in range(NPAIR)]
        kt_sb = [attn_pool.tile([P, T], bf16, tag=f"kt{p}", name=f"kt{p}") for p in range(NPAIR)]
        vp_sb = [attn_pool.tile([P, H, HS + 1], bf16, tag=f"vp{s}", name=f"vp{s}") for s in range(NS)]
        yt_sb = [yt_pool.tile([P, TQ], bf16, tag=f"yt{p}", name=f"yt{p}") for p in range(NPAIR)]

        # ---------------- phase 1: QKV projections ----------------
        with tc.tile_pool(name="wqk", bufs=3) as wqk_pool, \
             tc.tile_pool(name="wv", bufs=2) as wv_pool, \
             tc.tile_pool(name="qkv_ps", bufs=3, space="PSUM") as qkv_ps:

            for p in range(NPAIR):
                wq_t = wqk_pool.tile([P, KC, P], bf16, tag="wqk")
                nc.sync.dma_start(
                    out=wq_t,
                    in_=wq[:, p * P:(p + 1) * P].rearrange("(kc pp) m -> pp kc m", pp=P),
                )
                for t in range(NTQ):
                    ps = qkv_ps.tile([P, 512], f32, tag="ps")
                    for kc in range(KC):
                        nc.tensor.matmul(
                            ps,
                            wq_t[:, kc, :],
                            xkv_sb[:, kc, TQ + t * 512: TQ + (t + 1) * 512],
                            start=(kc == 0),
                            stop=(kc == KC - 1),
                        )
                    nc.vector.tensor_copy(qt_sb[p][:, t * 512:(t + 1) * 512], ps)

                wk_t = wqk_pool.tile([P, KC, P], bf16, tag="wqk")
                nc.sync.dma_start(
                    out=wk_t,
                    in_=wk[:, p * P:(p + 1) * P].rearrange("(kc pp) m -> pp kc m", pp=P),
                )
                for t in range(T // 512):
                    ps = qkv_ps.tile([P, 512], f32, tag="ps")
                    for kc in range(KC):
                        nc.tensor.matmul(
                            ps,
                            wk_t[:, kc, :],
                            xkv_sb[:, kc, t * 512:(t + 1) * 512],
                            start=(kc == 0),
                            stop=(kc == KC - 1),
                        )
                    nc.vector.tensor_copy(kt_sb[p][:, t * 512:(t + 1) * 512], ps)

            # V' with ones column: V-proj writes [s, 8 heads * 64] chunks
            for si in range(NS):
                nc.vector.memset(vp_sb[si][:, :, HS:HS + 1], 1.0)
            for half in range(2):
                wv_t = wv_pool.tile([P, KC, 512], bf16, tag="wv")
                nc.sync.dma_start(
                    out=wv_t,
                    in_=wv[:, half * 512:(half + 1) * 512].rearrange(
                        "(kc p) n -> p kc n", p=P
                    ),
                )
                for si in range(NS):
                    ps = qkv_ps.tile([P, 512], f32, tag="ps")
                    for kc in range(KC):
                        nc.tensor.matmul(
                            ps,
                            xkv_sb[:, kc, si * P:(si + 1) * P],
                            wv_t[:, kc, :],
                            start=(kc == 0),
                            stop=(kc == KC - 1),
                        )
                    dst = vp_sb[si][:, half * 8:(half + 1) * 8, 0:HS]
                    nc.vector.tensor_copy(dst, ps.rearrange("p (h d) -> p h d", h=8))

        # ---------------- phase 2: attention ----------------
        with tc.tile_pool(name="s_ps", bufs=3, space="PSUM") as s_ps, \
             tc.tile_pool(name="y_ps", bufs=2, space="PSUM") as y_ps, \
             tc.tile_pool(name="expt", bufs=4) as expt_pool, \
             tc.tile_pool(name="norm", bufs=3) as norm_pool:

            for t in range(NTQ):
                diag_lo = 8 + 4 * t  # s-tiles [diag_lo, diag_lo+4) are diagonal
                for p in range(NPAIR):
                    for hh in range(2):
                        h = 2 * p + hh
                        ps_y = y_ps.tile([HS + 1, 512], f32, tag="ps_y")
                        for si in range(NS):
                            ps_s = s_ps.tile([P, 512], f32, tag="ps_s")
                            nc.tensor.matmul(
                                ps_s,
                                kt_sb[p][hh * HS:(hh + 1) * HS, si * P:(si + 1) * P],
                                qt_sb[p][hh * HS:(hh + 1) * HS, t * 512:(t + 1) * 512],
                            )
                            et = expt_pool.tile([P, 512], bf16, tag="expt")
                            nc.scalar.activation(
                                et, ps_s, Exp,
                                bias=kb_sb[:, t, si:si + 1],
                                scale=SCALE,
                            )
                            if diag_lo <= si < diag_lo + 4:
                                nc.vector.tensor_mul(et, et, tri_sb[:, si - diag_lo, :])
                            nc.tensor.matmul(
                                ps_y,
                                vp_sb[si][:, h, :],
                                et,
                                start=(si == 0),
                                stop=(si == NS - 1),
                            )
                        # normalize: recip of denom row, broadcast across 64 partitions
                        rec1 = norm_pool.tile([1, 512], f32, tag="rec1")
                        nc.vector.reciprocal(rec1, ps_y[64:65, :])
                        rb = norm_pool.tile([HS, 512], f32, tag="rb")
                        nc.gpsimd.partition_broadcast(rb, rec1)
                        nc.vector.tensor_mul(
                            yt_sb[p][hh * HS:(hh + 1) * HS, t * 512:(t + 1) * 512],
                            ps_y[0:HS, :],
                            rb,
                        )

        xkv_ctx.close()  # free xkv SBUF

        # ---------------- phase 3: proj + residual -> x2^T ----------------
        late = ctx.enter_context(tc.tile_pool(name="late", bufs=1, side="right"))
        x2t_sb = late.tile([P, NCO, TQ], f32, tag="x2t")
        x2tb_sb = late.tile([P, NCO, TQ], bf16, tag="x2tb")

        with tc.tile_pool(name="wp", bufs=3) as wp_pool, \
             tc.tile_pool(name="pr_ps", bufs=3, space="PSUM") as pr_ps, \
             tc.tile_pool(name="xres_p", bufs=1) as xres_pool, \
             tc.tile_pool(name="bias_p", bufs=1) as bias_pool:

            xres_sb = xres_pool.tile([P, KC, TQ], f32, tag="xres")
            nc.sync.dma_start(out=xres_sb, in_=xres)

            for co in range(NCO):
                wp_t = wp_pool.tile([P, KC, P], bf16, tag="wp")
                nc.sync.dma_start(
                    out=wp_t,
                    in_=wproj[:, co * P:(co + 1) * P].rearrange(
                        "(kc p) m -> p kc m", p=P
                    ),
                )
                for t in range(NTQ):
                    ps = pr_ps.tile([P, 512], f32, tag="ps")
                    for p in range(NPAIR):
                        nc.tensor.matmul(
                            ps,
                            wp_t[:, p, :],
                            yt_sb[p][:, t * 512:(t + 1) * 512],
                            start=(p == 0),
                            stop=(p == NPAIR - 1),
                        )
                    sl = slice(t * 512, (t + 1) * 512)
                    # x2 = proj + (x + bproj)  (bproj folded into xres on host)
                    nc.vector.tensor_add(x2t_sb[:, co, sl], ps, xres_sb[:, co, sl])
                    nc.scalar.activation(x2tb_sb[:, co, sl], x2t_sb[:, co, sl], Copy)

        yt_ctx.close()    # free yt SBUF
        attn_ctx.close()  # free qt/kt/vp SBUF

        # ---------------- phase 4: FFN1 (h1^T = relu(W1^T x2 + b1)) ----------------
        h1_pool = ctx.enter_context(tc.tile_pool(name="h1_pool", bufs=1, side="right"))
        h1_sb = [h1_pool.tile([P, TQ], bf16, tag=f"h1_{f}", name=f"h1_{f}") for f in range(NF)]

        with tc.tile_pool(name="w1p", bufs=3) as w1_pool, \
             tc.tile_pool(name="f1_ps", bufs=3, space="PSUM") as f1_ps, \
             tc.tile_pool(name="bias1", bufs=1) as bias1_pool:

            b1_sb = bias1_pool.tile([P, NF], f32, tag="b1")
            nc.sync.dma_start(out=b1_sb, in_=b1)

            for f in range(NF):
                w1_t = w1_pool.tile([P, KC, P], bf16, tag="w1")
                nc.sync.dma_start(
                    out=w1_t,
                    in_=w1[:, f * P:(f + 1) * P].rearrange("(kc p) m -> p kc m", p=P),
                )
                for t in range(NTQ):
                    ps = f1_ps.tile([P, 512], f32, tag="ps")
                    for kc in range(KC):
                        nc.tensor.matmul(
                            ps,
                            w1_t[:, kc, :],
                            x2tb_sb[:, kc, t * 512:(t + 1) * 512],
                            start=(kc == 0),
                            stop=(kc == KC - 1),
                        )
                    nc.scalar.activation(
                        h1_sb[f][:, t * 512:(t + 1) * 512], ps, Relu,
                        bias=b1_sb[:, f:f + 1],
                    )

        # ---------------- phase 5: FFN2 + residual -> out^T ----------------
        with tc.tile_pool(name="w2p", bufs=2) as w2_pool, \
             tc.tile_pool(name="f2_ps", bufs=3, space="PSUM") as f2_ps, \
             tc.tile_pool(name="bias2", bufs=1) as bias2_pool, \
             tc.tile_pool(name="outst", bufs=3) as out_pool:

            b2_sb = bias2_pool.tile([P, NCO], f32, tag="b2")
            nc.sync.dma_start(out=b2_sb, in_=b2)

            for co in range(NCO):
                w2_t = w2_pool.tile([P, NF, P], bf16, tag="w2")
                nc.sync.dma_start(
                    out=w2_t,
                    in_=w2[:, co * P:(co + 1) * P].rearrange("(kf p) m -> p kf m", p=P),
                )
                for t in range(NTQ):
                    ps = f2_ps.tile([P, 512], f32, tag="ps")
                    for kf in range(NF):
                        nc.tensor.matmul(
                            ps,
                            w2_t[:, kf, :],
                            h1_sb[kf][:, t * 512:(t + 1) * 512],
                            start=(kf == 0),
                            stop=(kf == NF - 1),
                        )
                    ot = out_pool.tile([P, 512], f32, tag="ot")
                    nc.vector.tensor_add(ot, ps, x2t_sb[:, co, t * 512:(t + 1) * 512])
                    nc.vector.tensor_scalar_add(ot, ot, b2_sb[:, co:co + 1])
                    nc.sync.dma_start(
                        out=out[co * P:(co + 1) * P, t * 512:(t + 1) * 512],
                        in_=ot,
                    )


def prepare_core_inputs(inputs):
    """Host-side shard prep. Returns (stacked per-core arrays, replicated arrays)."""
    x = np.asarray(inputs["x"], np.float32)
    Wq = np.asarray(inputs["Wq"], np.float32)
    Wk = np.asarray(inputs["Wk"], np.float32)
    Wv = np.asarray(inputs["Wv"], np.float32)
    Wproj = np.asarray(inputs["Wproj"], np.float32)
    bproj = np.asarray(inputs["bproj"], np.float32)
    W1 = np.asarray(inputs["W1"], np.float32)
    b1 = np.asarray(inputs["b1"], np.float32)
    W2 = np.asarray(inputs["W2"], np.float32)
    b2 = np.asarray(inputs["b2"], np.float32)

    xkv_l, xres_l, kb_l = [], [], []
    for core in range(NCORE):
        b, j = divmod(core, 2)
        own = x[b, j * TQ:(j + 1) * TQ]        # [TQ, C]
        other = x[b, (1 - j) * TQ:(2 - j) * TQ]
        xkv = np.concatenate([other, own], axis=0).T  # [C, T], own half last
        xkv_l.append(np.ascontiguousarray(xkv).astype(BF16))
        xr = own.T + bproj[:, None]  # fold bproj into the residual input
        xres_l.append(
            np.ascontiguousarray(xr.reshape(KC, P, TQ).transpose(1, 0, 2)).astype(np.float32)
        )
        kb = np.zeros((P, NTQ, NS), np.float32)
        if j == 0:
            kb[:, :, 0:8] = NEG  # context half holds future tokens
        for t in range(NTQ):
            kb[:, t, 8 + 4 * (t + 1):NS] = NEG  # own-half above-diagonal tiles
        kb_l.append(kb)

    def headmaj(w):  # [H, C, HS] -> [C, H*HS]
        return np.ascontiguousarray(w.transpose(1, 0, 2).reshape(C, C)).astype(BF16)

    def biascols(v, n):  # [n*128] -> [128, n]
        return np.ascontiguousarray(v.reshape(n, P).T.astype(np.float32))

    percore = dict(
        xkv=np.stack(xkv_l),          # [8, C, T] bf16
        xres=np.stack(xres_l),        # [8, P, KC, TQ] f32
        kbias=np.stack(kb_l),         # [8, P, NTQ, NS] f32
    )
    repl = dict(
        wq=headmaj(Wq), wk=headmaj(Wk), wv=headmaj(Wv),
        wproj=np.ascontiguousarray(Wproj).astype(BF16),
        w1=np.ascontiguousarray(W1).astype(BF16),
        w2=np.ascontiguousarray(W2).astype(BF16),
        bproj=biascols(bproj, NCO), b1=biascols(b1, NF), b2=biascols(b2, NCO),
    )
    return percore, repl


@functools.lru_cache(maxsize=1)
def _get_fn():
    import jax
    from jax.sharding import Mesh, PartitionSpec
    import concourse.mybir as mybir
    from concourse.bass2jax import bass_jit, bass_shard_map

    @bass_jit
    def ker(nc, xkv, xres, kbias, wq, wk, wv, wproj, w1, w2, bproj, b1, b2):
        out = nc.dram_tensor("outT", [C, TQ], mybir.dt.float32, kind="ExternalOutput")
        build_body(nc, xkv[:], xres[:], kbias[:], wq[:], wk[:], wv[:], wproj[:],
                   w1[:], w2[:], bproj[:], b1[:], b2[:], out[:])
        return out

    devices = jax.devices()[:NCORE]
    mesh = Mesh(np.asarray(devices), ("core",))
    core = PartitionSpec("core")
    rep = PartitionSpec()
    fn = bass_shard_map(
        ker,
        mesh=mesh,
        in_specs=(core, core, core) + (rep,) * 9,
        out_specs=core,
    )
    return fn


def kernel(**inputs) -> np.ndarray:
    import jax

    percore, repl = prepare_core_inputs(inputs)
    fn = _get_fn()
    args = (
        percore["xkv"].reshape(NCORE * C, T),
        percore["xres"].reshape(NCORE * P, KC, TQ),
        percore["kbias"].reshape(NCORE * P, NTQ, NS),
        repl["wq"], repl["wk"], repl["wv"], repl["wproj"],
        repl["w1"], repl["w2"], repl["bproj"], repl["b1"], repl["b2"],
    )
    outT = np.asarray(jax.block_until_ready(fn(*args)))  # [8*C, TQ]
    outT = outT.reshape(NCORE, C, TQ)
    out = np.empty((B, T, C), np.float32)
    for core in range(NCORE):
        b, j = divmod(core, 2)
        out[b, j * TQ:(j + 1) * TQ] = outT[core].T
    return out
